# revision 4
# baseline (speedup 1.0000x reference)
"""Trainium2 Bass kernel for BaselineMultiStepRNN — split-fp32r edition.

Math (per original reference, 1-based step index t = 1..T):
    h_t   = tanh(Wx x_t + Wc cap_{t-1} + Whh h_{t-1} + b_ih + b_hh)
    drop_t = fc_w h_t + fc_b
    cap_t = cap_{t-1} - drop_t ;  out[:, t-1] = cap_t

Folded form used on device (state v_t = cap_t - fc_b):
    W'  = Whh - outer(Wc, fc_w)     (removes cap's one-step feedback lag)
    pre_t = Wx x_t + Wc v_{t-2} + W' h_{t-1}   (+ bias b via ACT bias port)
    h_t  = tanh(pre_t + b)
    d_t  = fc_w h_t
    v_t  = (v_{t-1} - fcb) - d_t          (v_0 = cap_0 - fcb, v_{-1} = cap_0)
    out[:, t-1] = v_t + fc_b

Precision: every matmul runs in float32r (1 cycle/row vs fp32's 4) using a
3-term hi/lo mantissa split that recovers fp32-grade accuracy.  TRN2's f32r
stores 11 mantissa bits (round-to-nearest on engine writes; operands with
<=11 mantissa bits pass through products exactly; PSUM accumulates fp32).
For every product A·B we compute Ahi·Bhi + Ahi·Blo + Alo·Bhi with
hi = trunc11(A), lo = A - hi; the dropped lo·lo term is O(2^-22) relative.
Measured on HW: a [128x128]@[128x256] split matmul lands at 1.9e-7 rel err
vs 1.7e-7 for native fp32.  This chaotic recurrence amplifies per-step noise
~3e5x, so plain f32r (1.7e-4/step) fails by ~100x while the split stays at
the fp32 envelope (tolerance 2e-2; numpy sim of this scheme: 1.8e-4).

Per step/core (batch slice BC=256), 22 matmuls all N=256 @1cyc/row:
  x-part  4: chunkA [vhi2|xhi63|xlo63] K=127, chunkB [vhi|xhi63|vlo] K=65
  recur  12: (Wphi,hhi) (Wphi,hlo) (Wplo,hhi) x 2 K-chunks x 2 out-halves
  fc      6: (fchi,hhi) (fchi,hlo) (fclo,hhi) x 2 K-chunks, M=1
h split: ACT writes tanh->fp32 and tanh->f32r (hhi, round-on-write); DVE
subtracts hlo.  v split: DVE stt (v), 2 rounding copies, 1 subtract — all
engine writes at 32-aligned base partitions (0/64), a TRN2 requirement.
"""

import os

os.environ.setdefault("MYCRO_LOCAL_CACHE", "1")

from contextlib import ExitStack

import numpy as np

import concourse.tile as tile
from concourse import bacc, mybir
from concourse.alu_op_type import AluOpType
from concourse.bass_utils import run_bass_kernel_spmd

T_FULL = 512
F = 63
H = 256
B_FULL = 2048
NCORES = 8
BC = B_FULL // NCORES  # 256 batch per core
CH = 8                 # time steps per x chunk tile
F32 = mybir.dt.float32
F32R = mybir.dt.float32r

KA = 2 * F + 1         # chunk A rows: vhi2(1) + xhi(63) + xlo(63) = 127
KB = F + 2             # chunk B rows: vhi(1) + xhi(63) + vlo(1) = 65

_CACHE: dict = {}


def _trunc11(x):
    u = np.ascontiguousarray(np.asarray(x, np.float32)).view(np.uint32)
    return (u & np.uint32(0xFFFFF000)).view(np.float32)


def _build(T: int):
    if T in _CACHE:
        return _CACHE[T]

    NSLOT = T + 2              # slot s holds step s+1's rows; +2 for v tail
    NCHUNK = (NSLOT + CH - 1) // CH
    nc = bacc.Bacc(
        "TRN2", target_bir_lowering=False, debug=False, enable_asserts=False
    )
    xAd = nc.dram_tensor("xA", [NCHUNK, KA, CH, BC], F32R, kind="ExternalInput").ap()
    xBd = nc.dram_tensor("xB", [NCHUNK, KB, CH, BC], F32R, kind="ExternalInput").ap()
    lhsAd = nc.dram_tensor("lhsA", [KA, 2, 128], F32R, kind="ExternalInput").ap()
    lhsBd = nc.dram_tensor("lhsB", [KB, 2, 128], F32R, kind="ExternalInput").ap()
    wphid = nc.dram_tensor("wphi", [128, 2, H], F32R, kind="ExternalInput").ap()
    wplod = nc.dram_tensor("wplo", [128, 2, H], F32R, kind="ExternalInput").ap()
    fchid = nc.dram_tensor("fchi", [128, 2], F32R, kind="ExternalInput").ap()
    fclod = nc.dram_tensor("fclo", [128, 2], F32R, kind="ExternalInput").ap()
    biasd = nc.dram_tensor("bias", [128, 2], F32, kind="ExternalInput").ap()
    fcbd = nc.dram_tensor("fcb", [1, 1], F32, kind="ExternalInput").ap()
    vind = nc.dram_tensor("vinit", [2, BC], F32, kind="ExternalInput").ap()
    voutd = nc.dram_tensor("vout", [T, 1, BC], F32, kind="ExternalOutput").ap()

    TANH = mybir.ActivationFunctionType.Tanh
    SUB = AluOpType.subtract

    with tile.TileContext(nc) as tc, ExitStack() as ctx:
        consts = ctx.enter_context(tc.tile_pool(name="consts", bufs=1))
        lhsA = consts.tile([KA, 2, 128], F32R)
        lhsB = consts.tile([KB, 2, 128], F32R)
        wphi = consts.tile([128, 2, H], F32R)
        wplo = consts.tile([128, 2, H], F32R)
        fchi = consts.tile([128, 2], F32R)
        fclo = consts.tile([128, 2], F32R)
        bias = consts.tile([128, 2], F32)
        fcb = consts.tile([1, 1], F32)
        vin1 = consts.tile([1, BC], F32)
        nc.sync.dma_start(lhsA[:], lhsAd[:])
        nc.sync.dma_start(lhsB[:], lhsBd[:])
        nc.sync.dma_start(wphi[:], wphid[:])
        nc.sync.dma_start(wplo[:], wplod[:])
        nc.sync.dma_start(fchi[:], fchid[:])
        nc.sync.dma_start(fclo[:], fclod[:])
        nc.sync.dma_start(bias[:], biasd[:])
        nc.sync.dma_start(fcb[:], fcbd[:])
        nc.sync.dma_start(vin1[:], vind[1:2, :])

        xapool = ctx.enter_context(tc.tile_pool(name="xapool", bufs=4))
        xbpool = ctx.enter_context(tc.tile_pool(name="xbpool", bufs=4))
        vlpool = ctx.enter_context(tc.tile_pool(name="vlpool", bufs=4))
        hpool = ctx.enter_context(tc.tile_pool(name="hpool", bufs=2))
        hsplit = ctx.enter_context(tc.tile_pool(name="hsplit", bufs=2))
        ppool = ctx.enter_context(tc.tile_pool(name="ppool", bufs=3, space="PSUM"))
        dpool = ctx.enter_context(tc.tile_pool(name="dpool", bufs=2, space="PSUM"))

        xatiles: dict = {}
        xbtiles: dict = {}

        def xachunk(c):
            if c not in xatiles:
                xt = xapool.tile([KA, CH, BC], F32R, name="xa", tag="xa")
                if c == 0:
                    nc.sync.dma_start(xt[:], xAd[c])
                else:
                    nc.sync.dma_start(xt[1:KA], xAd[c, 1:KA])
                xatiles[c] = xt
            return xatiles[c]

        def xbchunk(c):
            if c not in xbtiles:
                xt = xbpool.tile([KB, CH, BC], F32R, name="xb", tag="xb")
                if c == 0:
                    nc.sync.dma_start(xt[:], xBd[c])
                else:
                    nc.sync.dma_start(xt[1:F + 1], xBd[c, 1:F + 1])
                xbtiles[c] = xt
            return xbtiles[c]

        def slot_a(s):
            return xachunk(s // CH)[:, s % CH, :]

        def slot_b(s):
            return xbchunk(s // CH)[:, s % CH, :]

        def vrow_hi2(s):   # chunk A row 0 (pairs Wclo)
            return xachunk(s // CH)[0:1, s % CH, :]

        def vrow_hi(s):    # chunk B row 0 (pairs Wchi)
            return xbchunk(s // CH)[0:1, s % CH, :]

        def vrow_lo(s):    # chunk B row 64 (pairs Wchi)
            return xbchunk(s // CH)[F + 1:F + 2, s % CH, :]

        h_prev = None      # (hhi, hlo) tiles of step t-1, layout [128, 2*BC]
        d = None
        vf: dict = {}      # s -> [1, BC] fp32 tile with v_s

        def emit_vsub(s):
            """v_s = (v_{s-1} - fcb) - d_s; split rows into slot s+1."""
            prev = vin1[:] if s == 1 else vf[s - 1][:]
            v = vlpool.tile([1, BC], F32, name="v", tag="v")
            nc.vector.scalar_tensor_tensor(
                v[:], prev, fcb[0:1, 0:1], d[:], op0=SUB, op1=SUB
            )
            vf[s] = v
            vf.pop(s - 2, None)
            # hi rows (round-to-11 on F32R write): pairs Wchi and Wclo
            nc.vector.tensor_copy(vrow_hi(s + 1), v[:])
            nc.vector.tensor_copy(vrow_hi2(s + 1), v[:])
            # lo row: v - vhi
            nc.vector.tensor_tensor(
                vrow_lo(s + 1), v[:], vrow_hi(s + 1).bitcast(F32), SUB
            )
            nc.sync.dma_start(voutd[s - 1], v[:])

        for t in range(1, T + 1):
            hp = [
                ppool.tile([128, BC], F32, name="hp0", tag="hp0"),
                ppool.tile([128, BC], F32, name="hp1", tag="hp1"),
            ]
            first = h_prev is None
            ra = slot_a(t - 1)
            rb = slot_b(t - 1)
            # 1) x + capacity matmuls open the PSUM groups
            for mt in range(2):
                nc.tensor.matmul(
                    hp[mt][:], lhsA[:, mt, :], ra, start=True, stop=False,
                )
                nc.tensor.matmul(
                    hp[mt][:], lhsB[:, mt, :], rb, start=False, stop=first,
                )
            if not first:
                hhi, hlo = h_prev
                # 2) recurrent K0 terms (need first tanh half of t-1)
                for mt in range(2):
                    nc.tensor.matmul(
                        hp[mt][:], wphi[:, 0, mt * 128:(mt + 1) * 128],
                        hhi[:, 0:BC], start=False, stop=False,
                    )
                    nc.tensor.matmul(
                        hp[mt][:], wphi[:, 0, mt * 128:(mt + 1) * 128],
                        hlo[:, 0:BC], start=False, stop=False,
                    )
                    nc.tensor.matmul(
                        hp[mt][:], wplo[:, 0, mt * 128:(mt + 1) * 128],
                        hhi[:, 0:BC], start=False, stop=False,
                    )
                # 3) first-half fc terms for step t-1
                d = dpool.tile([1, BC], F32, name="d", tag="d")
                nc.tensor.matmul(d[:], fchi[:, 0:1], hhi[:, 0:BC],
                                 start=True, stop=False)
                nc.tensor.matmul(d[:], fchi[:, 0:1], hlo[:, 0:BC],
                                 start=False, stop=False)
                nc.tensor.matmul(d[:], fclo[:, 0:1], hhi[:, 0:BC],
                                 start=False, stop=False)
                # 4) recurrent K1 terms (need second tanh half of t-1)
                for mt in range(2):
                    nc.tensor.matmul(
                        hp[mt][:], wphi[:, 1, mt * 128:(mt + 1) * 128],
                        hhi[:, BC:2 * BC], start=False, stop=False,
                    )
                    nc.tensor.matmul(
                        hp[mt][:], wphi[:, 1, mt * 128:(mt + 1) * 128],
                        hlo[:, BC:2 * BC], start=False, stop=False,
                    )
                    nc.tensor.matmul(
                        hp[mt][:], wplo[:, 1, mt * 128:(mt + 1) * 128],
                        hhi[:, BC:2 * BC], start=False, stop=True,
                    )
                # 5) second-half fc terms for step t-1
                nc.tensor.matmul(d[:], fchi[:, 1:2], hhi[:, BC:2 * BC],
                                 start=False, stop=False)
                nc.tensor.matmul(d[:], fchi[:, 1:2], hlo[:, BC:2 * BC],
                                 start=False, stop=False)
                nc.tensor.matmul(d[:], fclo[:, 1:2], hhi[:, BC:2 * BC],
                                 start=False, stop=True)
            # 6) tanh halves (+ full-precision bias): fp32 h and f32r hhi on
            # ACT; hlo = h - hhi on DVE
            h = hpool.tile([128, 2 * BC], F32, name="h", tag="h")
            hhi_n = hsplit.tile([128, 2 * BC], F32R, name="hhi", tag="hhi")
            hlo_n = hsplit.tile([128, 2 * BC], F32R, name="hlo", tag="hlo")
            for half in range(2):
                sl = slice(half * BC, (half + 1) * BC)
                nc.scalar.activation(h[:, sl], hp[half][:], TANH,
                                     bias=bias[:, half:half + 1])
                nc.scalar.activation(hhi_n[:, sl], hp[half][:], TANH,
                                     bias=bias[:, half:half + 1])
                nc.vector.tensor_tensor(
                    hlo_n[:, sl], h[:, sl], hhi_n[:, sl].bitcast(F32), SUB
                )
            # 7) v update for step t-1
            if not first:
                emit_vsub(t - 1)
            h_prev = (hhi_n, hlo_n)

        # tail: fc + v update for step T
        hhi, hlo = h_prev
        d = dpool.tile([1, BC], F32, name="d", tag="d")
        nc.tensor.matmul(d[:], fchi[:, 0:1], hhi[:, 0:BC], start=True, stop=False)
        nc.tensor.matmul(d[:], fchi[:, 0:1], hlo[:, 0:BC], start=False, stop=False)
        nc.tensor.matmul(d[:], fclo[:, 0:1], hhi[:, 0:BC], start=False, stop=False)
        nc.tensor.matmul(d[:], fchi[:, 1:2], hhi[:, BC:2 * BC], start=False, stop=False)
        nc.tensor.matmul(d[:], fchi[:, 1:2], hlo[:, BC:2 * BC], start=False, stop=False)
        nc.tensor.matmul(d[:], fclo[:, 1:2], hhi[:, BC:2 * BC], start=False, stop=True)
        emit_vsub(T)

    nc.compile()
    _CACHE[T] = nc
    return nc


def _prep_maps(x_seq, seed_capacity, W_ih_w, W_ih_b, W_hh_w, W_hh_b, fc_w, fc_b, T):
    x_seq = np.asarray(x_seq, dtype=np.float32)
    seed = np.asarray(seed_capacity, dtype=np.float32).reshape(B_FULL)
    W_ih_w = np.asarray(W_ih_w, dtype=np.float32)
    W_ih_b = np.asarray(W_ih_b, dtype=np.float32)
    W_hh_w = np.asarray(W_hh_w, dtype=np.float32)
    W_hh_b = np.asarray(W_hh_b, dtype=np.float32)
    fc_w = np.asarray(fc_w, dtype=np.float32)
    fc_b = np.asarray(fc_b, dtype=np.float32)

    Wx = W_ih_w[:, :F]            # [H, 63]
    Wc = W_ih_w[:, F]             # [H]
    bvec = W_ih_b + W_hh_b        # [H]
    fcb_val = float(fc_b[0])
    Wp = (W_hh_w - np.outer(Wc, fc_w[0])).astype(np.float32)
    fc = fc_w[0]

    WxT = np.ascontiguousarray(Wx.T)               # [63, H]
    WxTh = _trunc11(WxT); WxTl = (WxT - WxTh).astype(np.float32)
    Wch = _trunc11(Wc); Wcl = (Wc - Wch).astype(np.float32)
    WpTh = _trunc11(Wp.T); WpTl = (Wp.T - WpTh).astype(np.float32)
    fch = _trunc11(fc); fcl = (fc - fch).astype(np.float32)

    # lhsA [KA=127, 2, 128]: row 0 Wclo (vhi2), 1..63 WxThi (xhi),
    # 64..126 WxThi (xlo)
    lhsA = np.zeros((KA, H), np.float32)
    lhsA[0] = Wcl
    lhsA[1:F + 1] = WxTh
    lhsA[F + 1:2 * F + 1] = WxTh
    lhsA = np.ascontiguousarray(lhsA.reshape(KA, 2, 128))
    # lhsB [KB=65, 2, 128]: row 0 Wchi (vhi), 1..63 WxTlo (xhi), 64 Wchi (vlo)
    lhsB = np.zeros((KB, H), np.float32)
    lhsB[0] = Wch
    lhsB[1:F + 1] = WxTl
    lhsB[F + 1] = Wch
    lhsB = np.ascontiguousarray(lhsB.reshape(KB, 2, 128))

    wphi = np.ascontiguousarray(WpTh.reshape(2, 128, H).transpose(1, 0, 2))
    wplo = np.ascontiguousarray(WpTl.reshape(2, 128, H).transpose(1, 0, 2))
    fchi = np.ascontiguousarray(fch.reshape(2, 128).T)      # [128, 2]
    fclo = np.ascontiguousarray(fcl.reshape(2, 128).T)
    biasm = np.ascontiguousarray(bvec.reshape(2, 128).T)    # [128, 2]
    fcb = np.array([[fcb_val]], dtype=np.float32)

    NSLOT = T + 2
    NCHUNK = (NSLOT + CH - 1) // CH

    in_maps = []
    for c in range(NCORES):
        sl = slice(c * BC, (c + 1) * BC)
        xc = x_seq[sl, :T, :]                                # [BC, T, F]
        xtr = np.ascontiguousarray(xc.transpose(1, 2, 0))    # [T, F, BC]
        Tp = NCHUNK * CH
        xtr = np.concatenate(
            [xtr, np.zeros((Tp - T, F, BC), np.float32)], axis=0
        )
        xch = xtr.reshape(NCHUNK, CH, F, BC).transpose(0, 2, 1, 3)
        xch_hi = _trunc11(xch)
        xch_lo = (xch - xch_hi).astype(np.float32)
        xA = np.zeros((NCHUNK, KA, CH, BC), np.float32)
        xA[:, 1:F + 1] = xch_hi
        xA[:, F + 1:2 * F + 1] = xch_lo
        xB = np.zeros((NCHUNK, KB, CH, BC), np.float32)
        xB[:, 1:F + 1] = xch_hi
        seedc = seed[sl]                                     # cap_0 = v_{-1}
        v0 = (seedc - fcb_val).astype(np.float32)
        for slot, vval in ((0, seedc), (1, v0)):
            vh = _trunc11(vval)
            xB[0, 0, slot] = vh                              # vhi (pairs Wchi)
            xB[0, F + 1, slot] = vval - vh                   # vlo
            xA[0, 0, slot] = vh                              # vhi2 (pairs Wclo)
        in_maps.append(
            {
                "xA": np.ascontiguousarray(xA),
                "xB": np.ascontiguousarray(xB),
                "lhsA": lhsA,
                "lhsB": lhsB,
                "wphi": wphi,
                "wplo": wplo,
                "fchi": fchi,
                "fclo": fclo,
                "bias": biasm,
                "fcb": fcb,
                "vinit": np.ascontiguousarray(np.stack([seedc, v0])),
            }
        )
    return in_maps, fcb_val


def _run(trace=False, **inputs):
    T = int(inputs.get("forecast_steps", T_FULL))
    nc = _build(T)
    in_maps, fcb_val = _prep_maps(
        inputs["x_seq"], inputs["seed_capacity"],
        inputs["W_ih_w"], inputs["W_ih_b"],
        inputs["W_hh_w"], inputs["W_hh_b"],
        inputs["fc_w"], inputs["fc_b"], T,
    )
    res = run_bass_kernel_spmd(
        nc, in_maps, core_ids=list(range(NCORES)), trace=trace
    )
    out = np.empty((B_FULL, T), np.float32)
    for c in range(NCORES):
        v = res.results[c]["vout"].reshape(T, BC)
        out[c * BC:(c + 1) * BC] = (v + fcb_val).T
    return out, res


def kernel(**inputs) -> np.ndarray:
    out, _ = _run(trace=False, **inputs)
    return out


# revision 6
# speedup vs baseline: 1.2224x; 1.2224x over previous
"""Trainium2 Bass kernel for BaselineMultiStepRNN — split-fp32r edition.

Math (per original reference, 1-based step index t = 1..T):
    h_t   = tanh(Wx x_t + Wc cap_{t-1} + Whh h_{t-1} + b_ih + b_hh)
    drop_t = fc_w h_t + fc_b
    cap_t = cap_{t-1} - drop_t ;  out[:, t-1] = cap_t

Folded form used on device (state v_t = cap_t - fc_b):
    W'  = Whh - outer(Wc, fc_w)     (removes cap's one-step feedback lag)
    pre_t = Wx x_t + Wc v_{t-2} + W' h_{t-1}   (+ bias b via ACT bias port)
    h_t  = tanh(pre_t + b)
    d_t  = fc_w h_t
    v_t  = (v_{t-1} - fcb) - d_t          (v_0 = cap_0 - fcb, v_{-1} = cap_0)
    out[:, t-1] = v_t + fc_b

Precision: every matmul runs in float32r (1 cycle/row vs fp32's 4) using a
3-term hi/lo mantissa split that recovers fp32-grade accuracy.  TRN2's f32r
stores 11 mantissa bits (round-to-nearest on engine writes; operands with
<=11 mantissa bits pass through products exactly; PSUM accumulates fp32).
For every product A·B we compute Ahi·Bhi + Ahi·Blo + Alo·Bhi with
hi = trunc11(A), lo = A - hi; the dropped lo·lo term is O(2^-22) relative.
Measured on HW: a [128x128]@[128x256] split matmul lands at 1.9e-7 rel err
vs 1.7e-7 for native fp32.  This chaotic recurrence amplifies per-step noise
~3e5x, so plain f32r (1.7e-4/step) fails by ~100x while the split stays at
the fp32 envelope (tolerance 2e-2; numpy sim of this scheme: 1.8e-4).

Per step/core (batch slice BC=256), 22 matmuls all N=256 @1cyc/row:
  x-part  4: chunkA [vhi2|xhi63|xlo63] K=127, chunkB [vhi|xhi63|vlo] K=65
  recur  12: (Wphi,hhi) (Wphi,hlo) (Wplo,hhi) x 2 K-chunks x 2 out-halves
  fc      6: (fchi,hhi) (fchi,hlo) (fclo,hhi) x 2 K-chunks, M=1
h split: ACT writes tanh->fp32 and tanh->f32r (hhi, round-on-write); DVE
subtracts hlo.  v split: DVE stt (v), 2 rounding copies, 1 subtract — all
engine writes at 32-aligned base partitions (0/64), a TRN2 requirement.
"""

import os

os.environ.setdefault("MYCRO_LOCAL_CACHE", "1")

from contextlib import ExitStack

import numpy as np

import concourse.tile as tile
from concourse import bacc, mybir
from concourse.alu_op_type import AluOpType
from concourse.bass_utils import run_bass_kernel_spmd

T_FULL = 512
F = 63
H = 256
B_FULL = 2048
NCORES = 8
BC = B_FULL // NCORES  # 256 batch per core
CH = 8                 # time steps per x chunk tile
F32 = mybir.dt.float32
F32R = mybir.dt.float32r

KA = 2 * F + 1         # chunk A rows: vhi2(1) + xhi(63) + xlo(63) = 127
KB = F + 2             # chunk B rows: vhi(1) + xhi(63) + vlo(1) = 65

_CACHE: dict = {}


def _trunc11(x):
    u = np.ascontiguousarray(np.asarray(x, np.float32)).view(np.uint32)
    return (u & np.uint32(0xFFFFF000)).view(np.float32)


def _build(T: int):
    if T in _CACHE:
        return _CACHE[T]

    NSLOT = T + 2              # slot s holds step s+1's rows; +2 for v tail
    NCHUNK = (NSLOT + CH - 1) // CH
    nc = bacc.Bacc(
        "TRN2", target_bir_lowering=False, debug=False, enable_asserts=False
    )
    xAd = nc.dram_tensor("xA", [NCHUNK, KA, CH, BC], F32R, kind="ExternalInput").ap()
    xBd = nc.dram_tensor("xB", [NCHUNK, KB, CH, BC], F32R, kind="ExternalInput").ap()
    lhsAd = nc.dram_tensor("lhsA", [KA, 2, 128], F32R, kind="ExternalInput").ap()
    lhsBd = nc.dram_tensor("lhsB", [KB, 2, 128], F32R, kind="ExternalInput").ap()
    wphid = nc.dram_tensor("wphi", [128, 2, H], F32R, kind="ExternalInput").ap()
    wplod = nc.dram_tensor("wplo", [128, 2, H], F32R, kind="ExternalInput").ap()
    fchid = nc.dram_tensor("fchi", [128, 2], F32R, kind="ExternalInput").ap()
    fclod = nc.dram_tensor("fclo", [128, 2], F32R, kind="ExternalInput").ap()
    biasd = nc.dram_tensor("bias", [128, 2], F32, kind="ExternalInput").ap()
    fcbd = nc.dram_tensor("fcb", [1, 1], F32, kind="ExternalInput").ap()
    vind = nc.dram_tensor("vinit", [2, BC], F32, kind="ExternalInput").ap()
    voutd = nc.dram_tensor("vout", [T, 1, BC], F32, kind="ExternalOutput").ap()

    TANH = mybir.ActivationFunctionType.Tanh
    SUB = AluOpType.subtract

    with tile.TileContext(nc) as tc, ExitStack() as ctx:
        consts = ctx.enter_context(tc.tile_pool(name="consts", bufs=1))
        lhsA = consts.tile([KA, 2, 128], F32R)
        lhsB = consts.tile([KB, 2, 128], F32R)
        wphi = consts.tile([128, 2, H], F32R)
        wplo = consts.tile([128, 2, H], F32R)
        fchi = consts.tile([128, 2], F32R)
        fclo = consts.tile([128, 2], F32R)
        bias = consts.tile([128, 2], F32)
        fcb = consts.tile([1, 1], F32)
        vin1 = consts.tile([1, BC], F32)
        nc.sync.dma_start(lhsA[:], lhsAd[:])
        nc.sync.dma_start(lhsB[:], lhsBd[:])
        nc.sync.dma_start(wphi[:], wphid[:])
        nc.sync.dma_start(wplo[:], wplod[:])
        nc.sync.dma_start(fchi[:], fchid[:])
        nc.sync.dma_start(fclo[:], fclod[:])
        nc.sync.dma_start(bias[:], biasd[:])
        nc.sync.dma_start(fcb[:], fcbd[:])
        nc.sync.dma_start(vin1[:], vind[1:2, :])

        xapool = ctx.enter_context(tc.tile_pool(name="xapool", bufs=4))
        xbpool = ctx.enter_context(tc.tile_pool(name="xbpool", bufs=4))
        vlpool = ctx.enter_context(tc.tile_pool(name="vlpool", bufs=4))
        hpool = ctx.enter_context(tc.tile_pool(name="hpool", bufs=2))
        hsplit = ctx.enter_context(tc.tile_pool(name="hsplit", bufs=2))
        ppool = ctx.enter_context(tc.tile_pool(name="ppool", bufs=3, space="PSUM"))
        dpool = ctx.enter_context(tc.tile_pool(name="dpool", bufs=2, space="PSUM"))

        xatiles: dict = {}
        xbtiles: dict = {}

        def xachunk(c):
            if c not in xatiles:
                xt = xapool.tile([KA, CH, BC], F32R, name="xa", tag="xa")
                if c == 0:
                    nc.sync.dma_start(xt[:], xAd[c])
                else:
                    nc.sync.dma_start(xt[1:KA], xAd[c, 1:KA])
                xatiles[c] = xt
            return xatiles[c]

        def xbchunk(c):
            if c not in xbtiles:
                xt = xbpool.tile([KB, CH, BC], F32R, name="xb", tag="xb")
                if c == 0:
                    nc.sync.dma_start(xt[:], xBd[c])
                else:
                    nc.sync.dma_start(xt[1:F + 1], xBd[c, 1:F + 1])
                xbtiles[c] = xt
            return xbtiles[c]

        def slot_a(s):
            return xachunk(s // CH)[:, s % CH, :]

        def slot_b(s):
            return xbchunk(s // CH)[:, s % CH, :]

        def vrow_hi2(s):   # chunk A row 0 (pairs Wclo)
            return xachunk(s // CH)[0:1, s % CH, :]

        def vrow_hi(s):    # chunk B row 0 (pairs Wchi)
            return xbchunk(s // CH)[0:1, s % CH, :]

        def vrow_lo(s):    # chunk B row 64 (pairs Wchi)
            return xbchunk(s // CH)[F + 1:F + 2, s % CH, :]

        h_prev = None      # (hhi, hlo) tiles of step t-1, layout [128, 2*BC]
        d = None
        vf: dict = {}      # s -> [1, BC] fp32 tile with v_s

        def emit_vsub(s):
            """v_s = (v_{s-1} - fcb) - d_s; split rows into slot s+1."""
            prev = vin1[:] if s == 1 else vf[s - 1][:]
            v = vlpool.tile([1, BC], F32, name="v", tag="v")
            nc.vector.scalar_tensor_tensor(
                v[:], prev, fcb[0:1, 0:1], d[:], op0=SUB, op1=SUB
            )
            vf[s] = v
            vf.pop(s - 2, None)
            # hi row (round-to-11 on F32R write), then lo = v - vhi; the
            # second hi copy (pairs Wclo, feeds xA) last — xA sits 6 matmuls
            # into the next step's PE stream, so it has the most slack
            nc.vector.tensor_copy(vrow_hi(s + 1), v[:])
            nc.vector.tensor_tensor(
                vrow_lo(s + 1), v[:], vrow_hi(s + 1).bitcast(F32), SUB
            )
            nc.vector.tensor_copy(vrow_hi2(s + 1), v[:])
            nc.sync.dma_start(voutd[s - 1], v[:])

        # Per-step PE stream (steady state), ordered so (a) hp[0]'s group
        # closes 8 matmuls in — its tanh overlaps the rest of the step's PE
        # work, hiding the ACT/DVE split latency — and (b) every matmul's
        # operand (hhi/hlo halves, v rows from the previous step's late DVE
        # chain) is ready well before the in-order PE queue reaches it:
        #   per mt: K0hi K1hi Wlo0 Wlo1 Whi-hlo0 xA Whi-hlo1 xB(stop)
        #   then fc (d of t-1): hhi0 hhi1 fclo-hhi0 fclo-hhi1 hlo0 hlo1
        for t in range(1, T + 1):
            hp = [
                ppool.tile([128, BC], F32, name="hp0", tag="hp0"),
                ppool.tile([128, BC], F32, name="hp1", tag="hp1"),
            ]
            first = h_prev is None
            ra = slot_a(t - 1)
            rb = slot_b(t - 1)
            h = hpool.tile([128, 2 * BC], F32, name="h", tag="h")
            hhi_n = hsplit.tile([128, 2 * BC], F32R, name="hhi", tag="hhi")
            hlo_n = hsplit.tile([128, 2 * BC], F32R, name="hlo", tag="hlo")
            if not first:
                hhi, hlo = h_prev
            for mt in range(2):
                o = hp[mt][:]
                ws = slice(mt * 128, (mt + 1) * 128)
                if not first:
                    nc.tensor.matmul(o, wphi[:, 0, ws], hhi[:, 0:BC],
                                     start=True, stop=False)
                    nc.tensor.matmul(o, wphi[:, 1, ws], hhi[:, BC:2 * BC],
                                     start=False, stop=False)
                    nc.tensor.matmul(o, wplo[:, 0, ws], hhi[:, 0:BC],
                                     start=False, stop=False)
                    nc.tensor.matmul(o, wplo[:, 1, ws], hhi[:, BC:2 * BC],
                                     start=False, stop=False)
                    nc.tensor.matmul(o, wphi[:, 0, ws], hlo[:, 0:BC],
                                     start=False, stop=False)
                    nc.tensor.matmul(o, lhsA[:, mt, :], ra,
                                     start=False, stop=False)
                    nc.tensor.matmul(o, wphi[:, 1, ws], hlo[:, BC:2 * BC],
                                     start=False, stop=False)
                    nc.tensor.matmul(o, lhsB[:, mt, :], rb,
                                     start=False, stop=True)
                else:
                    nc.tensor.matmul(o, lhsA[:, mt, :], ra,
                                     start=True, stop=False)
                    nc.tensor.matmul(o, lhsB[:, mt, :], rb,
                                     start=False, stop=True)
                # tanh of this half as soon as its group closes: f32r hhi
                # (round-on-write) first — it unblocks the next step's
                # hi-matmuls — then fp32 h, then hlo = h - hhi on DVE
                sl = slice(mt * BC, (mt + 1) * BC)
                nc.scalar.activation(hhi_n[:, sl], hp[mt][:], TANH,
                                     bias=bias[:, mt:mt + 1])
                nc.scalar.activation(h[:, sl], hp[mt][:], TANH,
                                     bias=bias[:, mt:mt + 1])
                nc.vector.tensor_tensor(
                    hlo_n[:, sl], h[:, sl], hhi_n[:, sl].bitcast(F32), SUB
                )
            if not first:
                # fc terms for step t-1, ordered by operand availability
                d = dpool.tile([1, BC], F32, name="d", tag="d")
                nc.tensor.matmul(d[:], fchi[:, 0:1], hhi[:, 0:BC],
                                 start=True, stop=False)
                nc.tensor.matmul(d[:], fchi[:, 1:2], hhi[:, BC:2 * BC],
                                 start=False, stop=False)
                nc.tensor.matmul(d[:], fclo[:, 0:1], hhi[:, 0:BC],
                                 start=False, stop=False)
                nc.tensor.matmul(d[:], fclo[:, 1:2], hhi[:, BC:2 * BC],
                                 start=False, stop=False)
                nc.tensor.matmul(d[:], fchi[:, 0:1], hlo[:, 0:BC],
                                 start=False, stop=False)
                nc.tensor.matmul(d[:], fchi[:, 1:2], hlo[:, BC:2 * BC],
                                 start=False, stop=True)
                emit_vsub(t - 1)
            h_prev = (hhi_n, hlo_n)

        # tail: fc + v update for step T
        hhi, hlo = h_prev
        d = dpool.tile([1, BC], F32, name="d", tag="d")
        nc.tensor.matmul(d[:], fchi[:, 0:1], hhi[:, 0:BC], start=True, stop=False)
        nc.tensor.matmul(d[:], fchi[:, 0:1], hlo[:, 0:BC], start=False, stop=False)
        nc.tensor.matmul(d[:], fclo[:, 0:1], hhi[:, 0:BC], start=False, stop=False)
        nc.tensor.matmul(d[:], fchi[:, 1:2], hhi[:, BC:2 * BC], start=False, stop=False)
        nc.tensor.matmul(d[:], fchi[:, 1:2], hlo[:, BC:2 * BC], start=False, stop=False)
        nc.tensor.matmul(d[:], fclo[:, 1:2], hhi[:, BC:2 * BC], start=False, stop=True)
        emit_vsub(T)

    nc.compile()
    _CACHE[T] = nc
    return nc


def _prep_maps(x_seq, seed_capacity, W_ih_w, W_ih_b, W_hh_w, W_hh_b, fc_w, fc_b, T):
    x_seq = np.asarray(x_seq, dtype=np.float32)
    seed = np.asarray(seed_capacity, dtype=np.float32).reshape(B_FULL)
    W_ih_w = np.asarray(W_ih_w, dtype=np.float32)
    W_ih_b = np.asarray(W_ih_b, dtype=np.float32)
    W_hh_w = np.asarray(W_hh_w, dtype=np.float32)
    W_hh_b = np.asarray(W_hh_b, dtype=np.float32)
    fc_w = np.asarray(fc_w, dtype=np.float32)
    fc_b = np.asarray(fc_b, dtype=np.float32)

    Wx = W_ih_w[:, :F]            # [H, 63]
    Wc = W_ih_w[:, F]             # [H]
    bvec = W_ih_b + W_hh_b        # [H]
    fcb_val = float(fc_b[0])
    Wp = (W_hh_w - np.outer(Wc, fc_w[0])).astype(np.float32)
    fc = fc_w[0]

    WxT = np.ascontiguousarray(Wx.T)               # [63, H]
    WxTh = _trunc11(WxT); WxTl = (WxT - WxTh).astype(np.float32)
    Wch = _trunc11(Wc); Wcl = (Wc - Wch).astype(np.float32)
    WpTh = _trunc11(Wp.T); WpTl = (Wp.T - WpTh).astype(np.float32)
    fch = _trunc11(fc); fcl = (fc - fch).astype(np.float32)

    # lhsA [KA=127, 2, 128]: row 0 Wclo (vhi2), 1..63 WxThi (xhi),
    # 64..126 WxThi (xlo)
    lhsA = np.zeros((KA, H), np.float32)
    lhsA[0] = Wcl
    lhsA[1:F + 1] = WxTh
    lhsA[F + 1:2 * F + 1] = WxTh
    lhsA = np.ascontiguousarray(lhsA.reshape(KA, 2, 128))
    # lhsB [KB=65, 2, 128]: row 0 Wchi (vhi), 1..63 WxTlo (xhi), 64 Wchi (vlo)
    lhsB = np.zeros((KB, H), np.float32)
    lhsB[0] = Wch
    lhsB[1:F + 1] = WxTl
    lhsB[F + 1] = Wch
    lhsB = np.ascontiguousarray(lhsB.reshape(KB, 2, 128))

    wphi = np.ascontiguousarray(WpTh.reshape(2, 128, H).transpose(1, 0, 2))
    wplo = np.ascontiguousarray(WpTl.reshape(2, 128, H).transpose(1, 0, 2))
    fchi = np.ascontiguousarray(fch.reshape(2, 128).T)      # [128, 2]
    fclo = np.ascontiguousarray(fcl.reshape(2, 128).T)
    biasm = np.ascontiguousarray(bvec.reshape(2, 128).T)    # [128, 2]
    fcb = np.array([[fcb_val]], dtype=np.float32)

    NSLOT = T + 2
    NCHUNK = (NSLOT + CH - 1) // CH

    in_maps = []
    for c in range(NCORES):
        sl = slice(c * BC, (c + 1) * BC)
        xc = x_seq[sl, :T, :]                                # [BC, T, F]
        xtr = np.ascontiguousarray(xc.transpose(1, 2, 0))    # [T, F, BC]
        Tp = NCHUNK * CH
        xtr = np.concatenate(
            [xtr, np.zeros((Tp - T, F, BC), np.float32)], axis=0
        )
        xch = xtr.reshape(NCHUNK, CH, F, BC).transpose(0, 2, 1, 3)
        xch_hi = _trunc11(xch)
        xch_lo = (xch - xch_hi).astype(np.float32)
        xA = np.zeros((NCHUNK, KA, CH, BC), np.float32)
        xA[:, 1:F + 1] = xch_hi
        xA[:, F + 1:2 * F + 1] = xch_lo
        xB = np.zeros((NCHUNK, KB, CH, BC), np.float32)
        xB[:, 1:F + 1] = xch_hi
        seedc = seed[sl]                                     # cap_0 = v_{-1}
        v0 = (seedc - fcb_val).astype(np.float32)
        for slot, vval in ((0, seedc), (1, v0)):
            vh = _trunc11(vval)
            xB[0, 0, slot] = vh                              # vhi (pairs Wchi)
            xB[0, F + 1, slot] = vval - vh                   # vlo
            xA[0, 0, slot] = vh                              # vhi2 (pairs Wclo)
        in_maps.append(
            {
                "xA": np.ascontiguousarray(xA),
                "xB": np.ascontiguousarray(xB),
                "lhsA": lhsA,
                "lhsB": lhsB,
                "wphi": wphi,
                "wplo": wplo,
                "fchi": fchi,
                "fclo": fclo,
                "bias": biasm,
                "fcb": fcb,
                "vinit": np.ascontiguousarray(np.stack([seedc, v0])),
            }
        )
    return in_maps, fcb_val


def _run(trace=False, **inputs):
    T = int(inputs.get("forecast_steps", T_FULL))
    nc = _build(T)
    in_maps, fcb_val = _prep_maps(
        inputs["x_seq"], inputs["seed_capacity"],
        inputs["W_ih_w"], inputs["W_ih_b"],
        inputs["W_hh_w"], inputs["W_hh_b"],
        inputs["fc_w"], inputs["fc_b"], T,
    )
    res = run_bass_kernel_spmd(
        nc, in_maps, core_ids=list(range(NCORES)), trace=trace
    )
    out = np.empty((B_FULL, T), np.float32)
    for c in range(NCORES):
        v = res.results[c]["vout"].reshape(T, BC)
        out[c * BC:(c + 1) * BC] = (v + fcb_val).T
    return out, res


def kernel(**inputs) -> np.ndarray:
    out, _ = _run(trace=False, **inputs)
    return out
